# revision 27
# baseline (speedup 1.0000x reference)
"""AncProbsLayer Trainium2 kernel — position-packed uint16 gather.

Computes anc[b, l, k*26+c] = P[b,k,token(b,l),c] where P[b,k] =
expm(tau_b * Q_k).

Host (tiny-parameter preprocessing, float64):
  Q_k -> symmetrized eigendecomposition, tau = softplus(tau_kernel)[idx],
  P[b,k] = U_k diag(exp(tau_b lam_k)) W_k, then per-sequence lookup
  tables quantized to 8-bit fixed point k = round(v*255) (rel-err of the
  full output vs fp64 reference: 4.6e-3, well inside the 2e-2 gate).

Key trick — position packing: one matmul output element carries TWO
consecutive positions.  The one-hot weights are fp16 with values
{0, 1, 256, 257}:

  lhsT[r, pp] = 256*[tok(2*pp)==r] + [tok(2*pp+1)==r]
  psum[pp, c] = sum_r lhsT[r, pp] * ktbl[r, c]
              = 256*k(2pp, c) + k(2pp+1, c)   (exact integer <= 65535)

The one-hots are fp16, not bf16: the value 257 (both positions share a
token) needs 9 significand bits, one more than bf16 has.

A single fp32->uint16 cast drains PSUM; the host splits high/low bytes
back into the even/odd positions.  Vs the bf16-output baseline this
halves ALL four hot resources at once: PE ldweights count (every
position-pair loads once instead of every position), PE moving columns,
PSUM->SBUF drain elements, and HBM output bytes (1 byte per value).

Device (pure data parallel, 64 of 512 sequences per NeuronCore):
  chunk = 128 position-pairs; out[128 pp, 208] = onehot_bf16[26,128]^T @
  ktbl_bf16[26, 208]; 4 chunks (one seq) per 2-bank PSUM tile; one
  fp32->u16 cast per seq alternating Vector/Scalar; uint16 output DMAs
  (1664 B contiguous per partition) issued from sync/gpsimd rings.
"""

import sys
import numpy as np

for _p in ("/opt/trn_rl_repo",):
    if _p not in sys.path:
        sys.path.insert(0, _p)

import ml_dtypes
import concourse.bass as bass
import concourse.tile as tile
from concourse import mybir
from concourse.bass_utils import run_bass_kernel_spmd
from concourse.vector_clock import ScopedClock

B, L, K, NR, S = 512, 1024, 8, 512, 20
EXT = 26
ROW = K * EXT          # 208 output row width
N_CORES = 8
B_SH = B // N_CORES    # 64 sequences per core
N_GRP = B_SH // 4      # 16 groups of 4 sequences
LP = L // 2            # 512 position-pairs per sequence
NCH = LP // 128        # 4 chunks of 128 position-pairs per sequence


def _patch_tile_drain():
    """This container's walrus rejects >1 sync-wait per instruction.  Split
    extra waits onto no-op instructions inserted just before, on the same
    engine (same program order on that engine => identical semantics)."""
    if getattr(tile.TileContext, "_drain_patched", False):
        return

    orig_lower = tile.TileContext._lower_ordered_insts

    def _split_lower(self, ordered):
        nc = self.nc
        for bb_name, insts in list(ordered.items()):
            new = []
            for inst in insts:
                si = getattr(inst, "sync_info", None)
                if si is not None and len(si.on_wait) > 1:
                    waits = list(si.on_wait)
                    for w in waits[:-1]:
                        nop = mybir.InstNoOp(
                            name=nc.get_next_instruction_name(),
                            ins=[], outs=[],
                            sync_info=mybir.SyncInfo(on_wait=[w], on_update=[]),
                            bass_nofuse=True,
                            engine=inst.engine,
                        )
                        new.append(nop)
                    inst.sync_info = mybir.SyncInfo(
                        on_wait=[waits[-1]], on_update=list(si.on_update)
                    )
                new.append(inst)
            ordered[bb_name] = new
        return orig_lower(self, ordered)

    tile.TileContext._lower_ordered_insts = _split_lower

    def _drain_and_barrier(self, tick_clock, wait_clock):
        nc = self.nc
        drain_inst = nc.sync.drain()
        wait_clock.add_sem_waits(
            drain_inst.ins, ScopedClock({None: tick_clock.global_clock})
        )
        si = drain_inst.ins.sync_info
        if si is not None and len(si.on_wait) > 1:
            waits = list(si.on_wait)
            drain_inst.ins.sync_info = mybir.SyncInfo(
                on_wait=[waits[0]], on_update=list(si.on_update)
            )
            for w in waits[1:]:
                d2 = nc.sync.drain()
                d2.ins.sync_info = mybir.SyncInfo(on_wait=[w], on_update=[])
        nc.all_engine_barrier()
        assert self.sems is not None
        popped = nc._tile_sem_poison_stack.pop()
        assert popped is self._sem_poison
        nc.clear_and_free_semaphores(list(self.sems.allocated().values()))
        nc.all_engine_barrier()

    tile.TileContext._drain_and_barrier = _drain_and_barrier
    tile.TileContext._drain_patched = True


def _softplus(x):
    return np.log1p(np.exp(-np.abs(x))) + np.maximum(x, 0.0)


def _host_prep(tau_kernel, exchangeability_kernel, frequencies, rate_indices):
    """Per-sequence 8-bit-quantized lookup tables (B, 26, 208) in float64.
    Row values are integers in [0, 255]; dequant is k/255 on the host."""
    E = exchangeability_kernel.astype(np.float64)
    freq = frequencies.astype(np.float64)
    eye = np.eye(S)
    R = _softplus(0.5 * (E + np.swapaxes(E, -1, -2))) * (1.0 - eye)
    Q = R * freq[None, None, :]
    diag = Q.sum(-1, keepdims=True)
    Q = Q - diag * eye
    mue = (freq[None, :, None] * diag).sum(-2, keepdims=True)
    Q = Q / np.maximum(mue, 1e-16)

    d = np.sqrt(freq)
    Sym = d[None, :, None] * Q / d[None, None, :]
    Sym = 0.5 * (Sym + np.swapaxes(Sym, -1, -2))
    lam, V = np.linalg.eigh(Sym)                       # (K,S), (K,S,S)
    U = V / d[:, None][None]                           # D^-1/2 V  (K, t, i)
    W = np.swapaxes(V, -1, -2) * d[None, None, :]      # V^T D^1/2 (K, i, c)

    tau = _softplus(tau_kernel.astype(np.float64))[
        np.asarray(rate_indices, dtype=np.int64)
    ]                                                   # (B,)
    e = np.exp(tau[:, None, None] * lam[None])          # (B, K, S)
    # P[b,k,t,c] = sum_i U[k,t,i] e[b,k,i] W[k,i,c]
    P = np.einsum("kti,bki,kic->bktc", U, e, W, optimize=True)

    tbl = np.zeros((B, EXT, ROW), np.float64)
    # std token t -> table row 6+t holds P[:, k, t, :] at cols k*26..k*26+19
    tbl[:, 6:EXT, :].reshape(B, S, K, EXT)[:, :, :, :S] = P.transpose(0, 2, 1, 3)
    # special token t (20..25) -> table row t-20 is one-hot at col k*26+t
    for s_ in range(EXT - S):
        for k in range(K):
            tbl[:, s_, k * EXT + S + s_] = 1.0
    kq = np.clip(np.round(tbl * 255.0), 0.0, 255.0)     # exact 8-bit ints
    return kq.astype(ml_dtypes.bfloat16)


def _make_in_maps(inputs, rate_indices, tau_kernel, exchangeability_kernel,
                  frequencies):
    tok = np.asarray(inputs, dtype=np.int64)
    # remap: std t -> 6+t (P rows), special t -> t-20 (one-hot rows)
    tok_r = np.where(tok < S, tok + (EXT - S), tok - S).astype(np.uint8)
    tbl = _host_prep(
        np.asarray(tau_kernel), np.asarray(exchangeability_kernel),
        np.asarray(frequencies), rate_indices,
    )
    # position-pair one-hots: pair pp_global = 4*p + c covers positions
    # (2*pp_global, 2*pp_global+1); weight = 256*[tok_even==r] +
    # [tok_odd==r], exact in bf16.  Group tile [128, 128, 4]: partition
    # 32*b4 + r is row r of seq 4g+b4; free (p, c); rows 26-31 zero.
    tokv = tok_r.reshape(B // 4, 4, 128, 4, 2)          # (grp, b4, p, c, e/o)
    r_ids = np.arange(32, dtype=np.uint8)[None, None, :, None, None]
    ohh = (
        256.0 * (tokv[:, :, None, :, :, 0] == r_ids)
        + 1.0 * (tokv[:, :, None, :, :, 1] == r_ids)
    )                                                   # (grp, b4, 32, p, c)
    ohh = ohh.reshape(B // 4, 128, 128 * 4).astype(np.float16)
    in_maps = []
    for core in range(N_CORES):
        sl = slice(core * B_SH, (core + 1) * B_SH)
        # table image: partition 32*b4 + r (r < 26) holds table row r of
        # seq 4*g + b4, free dims (g, 208); rows 26-31 unused
        dev = np.zeros((4, 32, N_GRP, ROW), dtype=ml_dtypes.bfloat16)
        dev[:, :EXT] = tbl[sl].reshape(N_GRP, 4, EXT, ROW).transpose(1, 2, 0, 3)
        in_maps.append({
            "tbl": dev.reshape(128, N_GRP, ROW),
            # partition-major so multi-group DMA slices are contiguous
            "ohd": np.ascontiguousarray(
                ohh[core * N_GRP : (core + 1) * N_GRP].transpose(1, 0, 2)),
        })
    return in_maps


def _build_bass():
    _patch_tile_drain()
    f32, bf16, u16 = mybir.dt.float32, mybir.dt.bfloat16, mybir.dt.uint16
    f16 = mybir.dt.float16

    nc = bass.Bass("TRN2", target_bir_lowering=False, debug=False,
                   num_devices=N_CORES)
    tbl_d = nc.declare_dram_parameter("tbl", [128, N_GRP, ROW], bf16,
                                      isOutput=False)
    ohd_d = nc.declare_dram_parameter("ohd", [128, N_GRP, 128 * 4], f16,
                                      isOutput=False)
    # quad-interleaved output layout: [quad, partition, seq-in-quad,
    # chunk, 208] makes each partition's DMA slice 6656 B contiguous
    # (1664 B chunks halve the per-engine DMA rate); the host reorders
    # axes during the byte-unpack it does anyway.
    out_d = nc.declare_dram_parameter("out", [N_GRP, 128, 16, ROW], u16,
                                      isOutput=True)

    with tile.TileContext(nc) as tc:
        with (
            tc.tile_pool(name="consts", bufs=1) as consts,
            tc.tile_pool(name="stage", bufs=6) as stagep,
            tc.tile_pool(name="ps", bufs=4, space="PSUM") as psp,
        ):
            # table tiles 4 groups per DMA (1664 B descriptors) and
            # one-hot tiles 2 groups per DMA (2048 B descriptors) —
            # small descriptors run the DMA engines at half rate.
            # Interleave in consumption order, alternating rings.
            # first tiles fine-grained (low latency to first matmul),
            # rest coarse (big descriptors keep the DMA engines at full
            # rate); one tile per load unit so early groups never wait
            # on later bytes (the tile tracker is per-tile), interleaved
            # in consumption order across both rings
            tbl_splits = [(0, 1), (1, 2), (2, 4)] + [
                (g, g + 4) for g in range(4, N_GRP, 4)]
            oh_splits = [(0, 1), (1, 2)] + [
                (g, g + 2) for g in range(2, N_GRP, 2)]
            t4_map = {}
            T4t = []
            for i, (g0, g1) in enumerate(tbl_splits):
                t = consts.tile([128, g1 - g0, ROW], bf16, name=f"t4_{i}")
                T4t.append(t)
                for g in range(g0, g1):
                    t4_map[g] = (t, g - g0)

            def t4_of(g):
                return t4_map[g]

            warm_in = consts.tile([128, 320], bf16)
            nc.vector.memset(warm_in, 0)

            ohs = {}
            OHt = []
            for i, (g0, g1) in enumerate(oh_splits):
                t = consts.tile([128, g1 - g0, 128, NCH], f16,
                                name=f"oh_{i}")
                OHt.append(t)
                for g in range(g0, g1):
                    ohs[g] = (t, g - g0)

            ring = [nc.gpsimd, nc.sync]
            nring = 0
            ti = oi = 0
            while ti < len(tbl_splits) or oi < len(oh_splits):
                if ti < len(tbl_splits):
                    g0, g1 = tbl_splits[ti]
                    ring[nring % 2].dma_start(
                        out=T4t[ti][:], in_=tbl_d[:, g0:g1, :])
                    nring += 1
                    ti += 1
                for _ in range(2):
                    if oi < len(oh_splits):
                        g0, g1 = oh_splits[oi]
                        ring[nring % 2].dma_start(
                            out=OHt[oi][:].rearrange("p g a b -> p (g a b)"),
                            in_=ohd_d[:, g0:g1, :],
                        )
                        nring += 1
                        oi += 1

            # PE pre-warm: a few dependency-free matmuls ramp the PE
            # p-state while the first loads land.  (More would delay the
            # real matmuls — they run serially on the PE queue.)
            for wi in range(4):
                wps = psp.tile([128, 4, 256], f32, tag="pst")
                nc.tensor.matmul(
                    wps[:].rearrange("p a b -> p (a b)")[:, 0:320],
                    lhsT=warm_in[:, 0:128], rhs=warm_in[:],
                    start=True, stop=True,
                )

            out_ap = out_d[:, :, :]
            for j in range(1, B_SH, 2):
                g = j // 4
                oh_cur, gi2 = ohs[g]
                T4, gi = t4_of(g)
                if j % 4 == 1:
                    stage = stagep.tile([128, 16, ROW], u16, tag="stage")
                soff = (j % 4 // 2) * 8
                # chunk c holds position-pairs 4*p + c so each partition's
                # 4 output rows are contiguous in DRAM (1664 B).  The two
                # seqs of a pair have their matmuls interleaved
                # (alternating PE row groups) so weight loads and matmuls
                # overlap in the array.
                pst = {}
                for jj in (j - 1, j):
                    pst[jj] = psp.tile([128, NCH, 256], f32, tag="pst",
                                       name=f"pst_{jj}")
                for c in range(NCH):
                    for jj in (j - 1, j):
                        b4 = jj % 4
                        nc.tensor.matmul(
                            pst[jj][:, c, 0:ROW],
                            lhsT=oh_cur[
                                b4 * 32 : b4 * 32 + EXT, gi2, :, c].squeeze(),
                            rhs=T4[b4 * 32 : b4 * 32 + EXT, gi, :],
                            start=True, stop=True,
                            tile_position=(b4 * 32, 0),
                        )
                # one fp32 -> u16 cast per seq; PSUM holds exact integers
                # <= 65535 so the cast is lossless.  Engines alternate per
                # seq so Vector and Scalar each drain half.
                for jj in (j - 1, j):
                    dst = stage[:, soff + (jj % 2) * 4 :
                                soff + (jj % 2) * 4 + NCH, :]
                    if (jj // 2) % 2 == 0:
                        eng_copy = (nc.vector.tensor_copy, nc.scalar.copy)
                    else:
                        eng_copy = (nc.scalar.copy, nc.vector.tensor_copy)
                    eng_copy[jj % 2](out=dst, in_=pst[jj][:, :, 0:ROW])
                # DMA per quad from the idle sync/gpsimd rings (copy
                # engines never wait on DMA issue); one 2D descriptor
                # set with 6656 B contiguous per partition.  The last
                # two quads drain on extra rings (scalar, plus a pair
                # split) so the tail shortens.
                if j % 4 == 3:
                    q4 = j // 4
                    if q4 == N_GRP - 1:
                        for h in range(2):
                            deng = nc.gpsimd if h == 0 else nc.sync
                            deng.dma_start(
                                out=bass.AP(
                                    tensor=out_ap.tensor,
                                    offset=(q4 * 128 * 16 + h * 8) * ROW,
                                    ap=[[16 * ROW, 128], [1, 8 * ROW]]),
                                in_=stage[:, 8 * h : 8 * h + 8, :]
                                .rearrange("p s j -> p (s j)"),
                            )
                        continue
                    deng = (nc.scalar if q4 in (5, 10, N_GRP - 2)
                            else nc.gpsimd if q4 % 2 == 0 else nc.sync)
                    deng.dma_start(
                        out=bass.AP(
                            tensor=out_ap.tensor,
                            offset=q4 * 128 * 16 * ROW,
                            ap=[[16 * ROW, 128], [1, 16 * ROW]]),
                        in_=stage[:].rearrange("p s j -> p (s j)"),
                    )
    return nc


_NC_CACHE = None


def kernel(inputs, rate_indices, tau_kernel, exchangeability_kernel,
           frequencies):
    global _NC_CACHE
    in_maps = _make_in_maps(inputs, rate_indices, tau_kernel,
                            exchangeability_kernel, frequencies)
    if _NC_CACHE is None:
        _NC_CACHE = _build_bass()
    nc = _NC_CACHE
    res = run_bass_kernel_spmd(nc, in_maps, core_ids=list(range(N_CORES)))
    packed = np.concatenate(
        [
            # device layout (quad, p, s, c, 208) -> (seq=4*quad+s,
            # pp=4*p+c, 208)
            np.asarray(res.results[c]["out"])
            .reshape(N_GRP, 128, 4, NCH, ROW)
            .transpose(0, 2, 1, 3, 4)
            .reshape(B_SH, LP, ROW)
            for c in range(N_CORES)
        ],
        axis=0,
    ).astype(np.uint16)                                  # (B, LP, 208)
    out = np.empty((B, L, ROW), np.float32)
    inv = np.float32(1.0 / 255.0)
    out[:, 0::2, :] = (packed >> 8).astype(np.float32) * inv
    out[:, 1::2, :] = (packed & 0xFF).astype(np.float32) * inv
    return out


if __name__ == "__main__":
    rng = np.random.default_rng(0)
    ins = {
        "inputs": rng.integers(0, EXT, size=(B, L)).astype(np.int32),
        "rate_indices": rng.integers(0, NR, size=(B,)).astype(np.int32),
        "tau_kernel": rng.standard_normal(NR).astype(np.float32),
        "exchangeability_kernel": rng.standard_normal((K, S, S)).astype(np.float32),
        "frequencies": rng.uniform(0.01, 1.0, S).astype(np.float32),
    }
    o = kernel(**ins)
    print("kernel out", o.shape, o.dtype)


# revision 28
# speedup vs baseline: 1.0058x; 1.0058x over previous
"""AncProbsLayer Trainium2 kernel — position-packed uint16 gather.

Computes anc[b, l, k*26+c] = P[b,k,token(b,l),c] where P[b,k] =
expm(tau_b * Q_k).

Host (tiny-parameter preprocessing, float64):
  Q_k -> symmetrized eigendecomposition, tau = softplus(tau_kernel)[idx],
  P[b,k] = U_k diag(exp(tau_b lam_k)) W_k, then per-sequence lookup
  tables quantized to 8-bit fixed point k = round(v*255) (rel-err of the
  full output vs fp64 reference: 4.6e-3, well inside the 2e-2 gate).

Key trick — position packing: one matmul output element carries TWO
consecutive positions.  The one-hot weights are fp16 with values
{0, 1, 256, 257}:

  lhsT[r, pp] = 256*[tok(2*pp)==r] + [tok(2*pp+1)==r]
  psum[pp, c] = sum_r lhsT[r, pp] * ktbl[r, c]
              = 256*k(2pp, c) + k(2pp+1, c)   (exact integer <= 65535)

The one-hots are fp16, not bf16: the value 257 (both positions share a
token) needs 9 significand bits, one more than bf16 has.

A single fp32->uint16 cast drains PSUM; the host splits high/low bytes
back into the even/odd positions.  Vs the bf16-output baseline this
halves ALL four hot resources at once: PE ldweights count (every
position-pair loads once instead of every position), PE moving columns,
PSUM->SBUF drain elements, and HBM output bytes (1 byte per value).

Device (pure data parallel, 64 of 512 sequences per NeuronCore):
  chunk = 128 position-pairs; out[128 pp, 208] = onehot_bf16[26,128]^T @
  ktbl_bf16[26, 208]; 4 chunks (one seq) per 2-bank PSUM tile; one
  fp32->u16 cast per seq alternating Vector/Scalar; uint16 output DMAs
  (1664 B contiguous per partition) issued from sync/gpsimd rings.
"""

import sys
import numpy as np

for _p in ("/opt/trn_rl_repo",):
    if _p not in sys.path:
        sys.path.insert(0, _p)

import ml_dtypes
import concourse.bass as bass
import concourse.tile as tile
from concourse import mybir
from concourse.bass_utils import run_bass_kernel_spmd
from concourse.vector_clock import ScopedClock

B, L, K, NR, S = 512, 1024, 8, 512, 20
EXT = 26
ROW = K * EXT          # 208 output row width
N_CORES = 8
B_SH = B // N_CORES    # 64 sequences per core
N_GRP = B_SH // 4      # 16 groups of 4 sequences
LP = L // 2            # 512 position-pairs per sequence
NCH = LP // 128        # 4 chunks of 128 position-pairs per sequence


def _patch_tile_drain():
    """This container's walrus rejects >1 sync-wait per instruction.  Split
    extra waits onto no-op instructions inserted just before, on the same
    engine (same program order on that engine => identical semantics)."""
    if getattr(tile.TileContext, "_drain_patched", False):
        return

    orig_lower = tile.TileContext._lower_ordered_insts

    def _split_lower(self, ordered):
        nc = self.nc
        for bb_name, insts in list(ordered.items()):
            new = []
            for inst in insts:
                si = getattr(inst, "sync_info", None)
                if si is not None and len(si.on_wait) > 1:
                    waits = list(si.on_wait)
                    for w in waits[:-1]:
                        nop = mybir.InstNoOp(
                            name=nc.get_next_instruction_name(),
                            ins=[], outs=[],
                            sync_info=mybir.SyncInfo(on_wait=[w], on_update=[]),
                            bass_nofuse=True,
                            engine=inst.engine,
                        )
                        new.append(nop)
                    inst.sync_info = mybir.SyncInfo(
                        on_wait=[waits[-1]], on_update=list(si.on_update)
                    )
                new.append(inst)
            ordered[bb_name] = new
        return orig_lower(self, ordered)

    tile.TileContext._lower_ordered_insts = _split_lower

    def _drain_and_barrier(self, tick_clock, wait_clock):
        nc = self.nc
        drain_inst = nc.sync.drain()
        wait_clock.add_sem_waits(
            drain_inst.ins, ScopedClock({None: tick_clock.global_clock})
        )
        si = drain_inst.ins.sync_info
        if si is not None and len(si.on_wait) > 1:
            waits = list(si.on_wait)
            drain_inst.ins.sync_info = mybir.SyncInfo(
                on_wait=[waits[0]], on_update=list(si.on_update)
            )
            for w in waits[1:]:
                d2 = nc.sync.drain()
                d2.ins.sync_info = mybir.SyncInfo(on_wait=[w], on_update=[])
        nc.all_engine_barrier()
        assert self.sems is not None
        popped = nc._tile_sem_poison_stack.pop()
        assert popped is self._sem_poison
        nc.clear_and_free_semaphores(list(self.sems.allocated().values()))
        nc.all_engine_barrier()

    tile.TileContext._drain_and_barrier = _drain_and_barrier
    tile.TileContext._drain_patched = True


def _softplus(x):
    return np.log1p(np.exp(-np.abs(x))) + np.maximum(x, 0.0)


def _host_prep(tau_kernel, exchangeability_kernel, frequencies, rate_indices):
    """Per-sequence 8-bit-quantized lookup tables (B, 26, 208) in float64.
    Row values are integers in [0, 255]; dequant is k/255 on the host."""
    E = exchangeability_kernel.astype(np.float64)
    freq = frequencies.astype(np.float64)
    eye = np.eye(S)
    R = _softplus(0.5 * (E + np.swapaxes(E, -1, -2))) * (1.0 - eye)
    Q = R * freq[None, None, :]
    diag = Q.sum(-1, keepdims=True)
    Q = Q - diag * eye
    mue = (freq[None, :, None] * diag).sum(-2, keepdims=True)
    Q = Q / np.maximum(mue, 1e-16)

    d = np.sqrt(freq)
    Sym = d[None, :, None] * Q / d[None, None, :]
    Sym = 0.5 * (Sym + np.swapaxes(Sym, -1, -2))
    lam, V = np.linalg.eigh(Sym)                       # (K,S), (K,S,S)
    U = V / d[:, None][None]                           # D^-1/2 V  (K, t, i)
    W = np.swapaxes(V, -1, -2) * d[None, None, :]      # V^T D^1/2 (K, i, c)

    tau = _softplus(tau_kernel.astype(np.float64))[
        np.asarray(rate_indices, dtype=np.int64)
    ]                                                   # (B,)
    e = np.exp(tau[:, None, None] * lam[None])          # (B, K, S)
    # P[b,k,t,c] = sum_i U[k,t,i] e[b,k,i] W[k,i,c]
    P = np.einsum("kti,bki,kic->bktc", U, e, W, optimize=True)

    tbl = np.zeros((B, EXT, ROW), np.float64)
    # std token t -> table row 6+t holds P[:, k, t, :] at cols k*26..k*26+19
    tbl[:, 6:EXT, :].reshape(B, S, K, EXT)[:, :, :, :S] = P.transpose(0, 2, 1, 3)
    # special token t (20..25) -> table row t-20 is one-hot at col k*26+t
    for s_ in range(EXT - S):
        for k in range(K):
            tbl[:, s_, k * EXT + S + s_] = 1.0
    kq = np.clip(np.round(tbl * 255.0), 0.0, 255.0)     # exact 8-bit ints
    return kq.astype(ml_dtypes.bfloat16)


def _make_in_maps(inputs, rate_indices, tau_kernel, exchangeability_kernel,
                  frequencies):
    tok = np.asarray(inputs, dtype=np.int64)
    # remap: std t -> 6+t (P rows), special t -> t-20 (one-hot rows)
    tok_r = np.where(tok < S, tok + (EXT - S), tok - S).astype(np.uint8)
    tbl = _host_prep(
        np.asarray(tau_kernel), np.asarray(exchangeability_kernel),
        np.asarray(frequencies), rate_indices,
    )
    # position-pair one-hots: pair pp_global = 4*p + c covers positions
    # (2*pp_global, 2*pp_global+1); weight = 256*[tok_even==r] +
    # [tok_odd==r], exact in bf16.  Group tile [128, 128, 4]: partition
    # 32*b4 + r is row r of seq 4g+b4; free (p, c); rows 26-31 zero.
    tokv = tok_r.reshape(B // 4, 4, 128, 4, 2)          # (grp, b4, p, c, e/o)
    r_ids = np.arange(32, dtype=np.uint8)[None, None, :, None, None]
    ohh = (
        256.0 * (tokv[:, :, None, :, :, 0] == r_ids)
        + 1.0 * (tokv[:, :, None, :, :, 1] == r_ids)
    )                                                   # (grp, b4, 32, p, c)
    ohh = ohh.reshape(B // 4, 128, 128 * 4).astype(np.float16)
    in_maps = []
    for core in range(N_CORES):
        sl = slice(core * B_SH, (core + 1) * B_SH)
        # table image: partition 32*b4 + r (r < 26) holds table row r of
        # seq 4*g + b4, free dims (g, 208); rows 26-31 unused
        dev = np.zeros((4, 32, N_GRP, ROW), dtype=ml_dtypes.bfloat16)
        dev[:, :EXT] = tbl[sl].reshape(N_GRP, 4, EXT, ROW).transpose(1, 2, 0, 3)
        in_maps.append({
            "tbl": dev.reshape(128, N_GRP, ROW),
            # partition-major so multi-group DMA slices are contiguous
            "ohd": np.ascontiguousarray(
                ohh[core * N_GRP : (core + 1) * N_GRP].transpose(1, 0, 2)),
        })
    return in_maps


def _build_bass():
    _patch_tile_drain()
    f32, bf16, u16 = mybir.dt.float32, mybir.dt.bfloat16, mybir.dt.uint16
    f16 = mybir.dt.float16

    nc = bass.Bass("TRN2", target_bir_lowering=False, debug=False,
                   num_devices=N_CORES)
    tbl_d = nc.declare_dram_parameter("tbl", [128, N_GRP, ROW], bf16,
                                      isOutput=False)
    ohd_d = nc.declare_dram_parameter("ohd", [128, N_GRP, 128 * 4], f16,
                                      isOutput=False)
    # quad-interleaved output layout: [quad, partition, seq-in-quad,
    # chunk, 208] makes each partition's DMA slice 6656 B contiguous
    # (1664 B chunks halve the per-engine DMA rate); the host reorders
    # axes during the byte-unpack it does anyway.
    out_d = nc.declare_dram_parameter("out", [N_GRP, 128, 16, ROW], u16,
                                      isOutput=True)

    with tile.TileContext(nc) as tc:
        with (
            tc.tile_pool(name="consts", bufs=1) as consts,
            tc.tile_pool(name="stage", bufs=6) as stagep,
            tc.tile_pool(name="ps", bufs=4, space="PSUM") as psp,
        ):
            # table tiles 4 groups per DMA (1664 B descriptors) and
            # one-hot tiles 2 groups per DMA (2048 B descriptors) —
            # small descriptors run the DMA engines at half rate.
            # Interleave in consumption order, alternating rings.
            # first tiles fine-grained (low latency to first matmul),
            # rest coarse (big descriptors keep the DMA engines at full
            # rate); one tile per load unit so early groups never wait
            # on later bytes (the tile tracker is per-tile), interleaved
            # in consumption order across both rings
            tbl_splits = [(0, 1), (1, 2), (2, 4)] + [
                (g, g + 4) for g in range(4, N_GRP, 4)]
            oh_splits = [(0, 1), (1, 2)] + [
                (g, g + 2) for g in range(2, N_GRP, 2)]
            t4_map = {}
            T4t = []
            for i, (g0, g1) in enumerate(tbl_splits):
                t = consts.tile([128, g1 - g0, ROW], bf16, name=f"t4_{i}")
                T4t.append(t)
                for g in range(g0, g1):
                    t4_map[g] = (t, g - g0)

            def t4_of(g):
                return t4_map[g]

            warm_in = consts.tile([128, 320], bf16)
            nc.vector.memset(warm_in, 0)

            ohs = {}
            OHt = []
            for i, (g0, g1) in enumerate(oh_splits):
                t = consts.tile([128, g1 - g0, 128, NCH], f16,
                                name=f"oh_{i}")
                OHt.append(t)
                for g in range(g0, g1):
                    ohs[g] = (t, g - g0)

            ring = [nc.gpsimd, nc.sync]
            nring = 0
            ti = oi = 0
            while ti < len(tbl_splits) or oi < len(oh_splits):
                if ti < len(tbl_splits):
                    g0, g1 = tbl_splits[ti]
                    ring[nring % 2].dma_start(
                        out=T4t[ti][:], in_=tbl_d[:, g0:g1, :])
                    nring += 1
                    ti += 1
                for _ in range(2):
                    if oi < len(oh_splits):
                        g0, g1 = oh_splits[oi]
                        ring[nring % 2].dma_start(
                            out=OHt[oi][:].rearrange("p g a b -> p (g a b)"),
                            in_=ohd_d[:, g0:g1, :],
                        )
                        nring += 1
                        oi += 1

            # PE pre-warm: a few dependency-free matmuls ramp the PE
            # p-state while the first loads land.  (More would delay the
            # real matmuls — they run serially on the PE queue.)
            for wi in range(4):
                wps = psp.tile([128, 4, 256], f32, tag="pst")
                nc.tensor.matmul(
                    wps[:].rearrange("p a b -> p (a b)")[:, 0:320],
                    lhsT=warm_in[:, 0:128], rhs=warm_in[:],
                    start=True, stop=True,
                )

            out_ap = out_d[:, :, :]
            for j in range(1, B_SH, 2):
                g = j // 4
                oh_cur, gi2 = ohs[g]
                T4, gi = t4_of(g)
                if j % 4 == 1:
                    stage = stagep.tile([128, 16, ROW], u16, tag="stage")
                soff = (j % 4 // 2) * 8
                # chunk c holds position-pairs 4*p + c so each partition's
                # 4 output rows are contiguous in DRAM (1664 B).  The two
                # seqs of a pair have their matmuls interleaved
                # (alternating PE row groups) so weight loads and matmuls
                # overlap in the array.
                pst = {}
                for jj in (j - 1, j):
                    pst[jj] = psp.tile([128, NCH, 256], f32, tag="pst",
                                       name=f"pst_{jj}")
                # seq A's chunks finish early (slot 5 of 8) so its copy
                # starts sooner and frees the PSUM tile before pair q+2
                # needs it; mostly alternating row groups keeps weight
                # loads overlapped with matmuls
                order = [(0, j - 1), (0, j), (1, j - 1), (1, j),
                         (2, j - 1), (3, j - 1), (2, j), (3, j)]
                for c, jj in order:
                    b4 = jj % 4
                    nc.tensor.matmul(
                        pst[jj][:, c, 0:ROW],
                        lhsT=oh_cur[
                            b4 * 32 : b4 * 32 + EXT, gi2, :, c].squeeze(),
                        rhs=T4[b4 * 32 : b4 * 32 + EXT, gi, :],
                        start=True, stop=True,
                        tile_position=(b4 * 32, 0),
                    )
                # one fp32 -> u16 cast per seq; PSUM holds exact integers
                # <= 65535 so the cast is lossless.  Engines alternate per
                # seq so Vector and Scalar each drain half.
                for jj in (j - 1, j):
                    dst = stage[:, soff + (jj % 2) * 4 :
                                soff + (jj % 2) * 4 + NCH, :]
                    if (jj // 2) % 2 == 0:
                        eng_copy = (nc.vector.tensor_copy, nc.scalar.copy)
                    else:
                        eng_copy = (nc.scalar.copy, nc.vector.tensor_copy)
                    eng_copy[jj % 2](out=dst, in_=pst[jj][:, :, 0:ROW])
                # DMA per quad from the idle sync/gpsimd rings (copy
                # engines never wait on DMA issue); one 2D descriptor
                # set with 6656 B contiguous per partition.  The last
                # two quads drain on extra rings (scalar, plus a pair
                # split) so the tail shortens.
                if j % 4 == 3:
                    q4 = j // 4
                    if q4 == N_GRP - 1:
                        for h in range(2):
                            deng = nc.gpsimd if h == 0 else nc.sync
                            deng.dma_start(
                                out=bass.AP(
                                    tensor=out_ap.tensor,
                                    offset=(q4 * 128 * 16 + h * 8) * ROW,
                                    ap=[[16 * ROW, 128], [1, 8 * ROW]]),
                                in_=stage[:, 8 * h : 8 * h + 8, :]
                                .rearrange("p s j -> p (s j)"),
                            )
                        continue
                    deng = (nc.scalar if q4 in (5, 10, N_GRP - 2)
                            else nc.gpsimd if q4 % 2 == 0 else nc.sync)
                    deng.dma_start(
                        out=bass.AP(
                            tensor=out_ap.tensor,
                            offset=q4 * 128 * 16 * ROW,
                            ap=[[16 * ROW, 128], [1, 16 * ROW]]),
                        in_=stage[:].rearrange("p s j -> p (s j)"),
                    )
    return nc


_NC_CACHE = None


def kernel(inputs, rate_indices, tau_kernel, exchangeability_kernel,
           frequencies):
    global _NC_CACHE
    in_maps = _make_in_maps(inputs, rate_indices, tau_kernel,
                            exchangeability_kernel, frequencies)
    if _NC_CACHE is None:
        _NC_CACHE = _build_bass()
    nc = _NC_CACHE
    res = run_bass_kernel_spmd(nc, in_maps, core_ids=list(range(N_CORES)))
    packed = np.concatenate(
        [
            # device layout (quad, p, s, c, 208) -> (seq=4*quad+s,
            # pp=4*p+c, 208)
            np.asarray(res.results[c]["out"])
            .reshape(N_GRP, 128, 4, NCH, ROW)
            .transpose(0, 2, 1, 3, 4)
            .reshape(B_SH, LP, ROW)
            for c in range(N_CORES)
        ],
        axis=0,
    ).astype(np.uint16)                                  # (B, LP, 208)
    out = np.empty((B, L, ROW), np.float32)
    inv = np.float32(1.0 / 255.0)
    out[:, 0::2, :] = (packed >> 8).astype(np.float32) * inv
    out[:, 1::2, :] = (packed & 0xFF).astype(np.float32) * inv
    return out


if __name__ == "__main__":
    rng = np.random.default_rng(0)
    ins = {
        "inputs": rng.integers(0, EXT, size=(B, L)).astype(np.int32),
        "rate_indices": rng.integers(0, NR, size=(B,)).astype(np.int32),
        "tau_kernel": rng.standard_normal(NR).astype(np.float32),
        "exchangeability_kernel": rng.standard_normal((K, S, S)).astype(np.float32),
        "frequencies": rng.uniform(0.01, 1.0, S).astype(np.float32),
    }
    o = kernel(**ins)
    print("kernel out", o.shape, o.dtype)
